# revision 33
# baseline (speedup 1.0000x reference)
"""Multi-head attention (B=2, N=2048, C=1024, H=16, qk-RMSNorm) on 8 TRN2 cores.

Sharding: tensor-parallel over heads x data-parallel over batch.
Core cid handles batch b = cid // 4 and head group g = cid % 4 (4 heads,
c_local = 256 channels). Each core computes qkv for its heads, per-head
RMSNorm on q/k, full softmax attention for its (b, heads), and a partial
output projection against its 256 rows of w_proj. The host sums the 4
partials per batch (TP unshard), adds b_proj and the v-bias contribution
(softmax weights sum to 1, so b_v passes through attention additively:
out += b_v @ w_proj), and stacks the 2 batches.

Precision plan (rel-err budget 2e-2, measured ~1e-2):
- weights / x / v / probs / attnT / output partials: bf16 (halves DMA+SBUF,
  enables fast-weight-load on the PE; matmul MAC rate is dtype-independent)
- q/k kept fp32r through RMSNorm and the S matmul (logit noise dominates
  the error budget otherwise)
- softmax exp: split between the Scalar engine (table exp) and the Vector
  engine (Schraudolph bit-trick: y*A+B -> int16 round -> bitcast bf16,
  ~3% max rel err on probs) so neither engine starves the PE. The ones
  column appended to V gives denominators from the same AV matmuls; the
  denominator uses the same approximated exps, so common-mode error
  cancels.

Engine balance per attention unit (2 heads x 512 queries, 16 key blocks):
PE ~12us (S pairs on alternating 64-row groups run concurrently; AV M=65;
1 broadcast matmul; projection share), ACT ~12us (12 exp tiles), DVE ~10us
(4 exp tiles + normalization chain + PSUM evacuations). Keeping every
engine under the PE's per-unit time keeps the PE busy and the HAM clock
gate at 8/8 (the fp32 baseline lost ~2x to ACT-starved PE idle windows).
"""

import sys

if "/opt/trn_rl_repo" not in sys.path:
    sys.path.insert(0, "/opt/trn_rl_repo")

from contextlib import ExitStack

import numpy as np
import ml_dtypes

import concourse.mybir as mybir
import concourse.tile as tile
from concourse import bacc
from concourse.bass_utils import run_bass_kernel_spmd

F32 = mybir.dt.float32
F32R = mybir.dt.float32r
BF16 = mybir.dt.bfloat16
I16 = mybir.dt.int16
AF = mybir.ActivationFunctionType
ALU = mybir.AluOpType

# Problem constants (hardcoded per contract)
B, N, C, H = 2, 2048, 1024, 16
D = C // H          # 64
EPS = 1e-6
NCORES = 8
GROUPS = 4          # head groups (cores per batch)
HL = H // GROUPS    # heads per core = 4
CL = HL * D         # local channels = 256
SCALE = D ** -0.5   # 0.125

# Tiling
P = 128             # partition dim
KT = C // P         # 8 contraction tiles over C
NQ = 512            # query-block (free dim of S^T / AV matmuls)
NB = N // P         # 16 key blocks of 128
HPB = P // D        # heads per 128-channel block = 2
VW = D + 1          # 65: v columns + ones column

# Schraudolph exp via int16: bf16(exp(y)) ~= int16(y*A16 + B16) bitcast bf16
A16 = (2.0 ** 23 / np.log(2.0)) / 65536.0   # 184.6650...
B16 = 16256.0 - 334500.0 / 65536.0          # tuned for minimax rel err ~3.3%
# which key blocks (mod 4) go to the DVE exp path; rest go to ACT.
# position 1 of each burst: the ring-stalled S matmuls i+2, i+3 then wait on
# exps draining on two different engines instead of queueing behind one
DVE_EXP_MOD = (1,)


def build(n=N, nq=NQ):
    """Build the SPMD Bass module. n = sequence length (for scaled tests)."""
    nb = n // P
    nj = n // nq
    kt = KT

    nc = bacc.Bacc("TRN2", target_bir_lowering=False, debug=False,
                   num_devices=NCORES)

    xT_d = nc.dram_tensor("xT", [C, n], BF16, kind="ExternalInput").ap()
    wqk_d = nc.dram_tensor("w_qk", [C, 2 * CL], BF16, kind="ExternalInput").ap()
    wv_d = nc.dram_tensor("w_v", [C, CL], BF16, kind="ExternalInput").ap()
    wpr_d = nc.dram_tensor("w_pr", [CL, C], BF16, kind="ExternalInput").ap()
    bqk_d = nc.dram_tensor("b_qk", [P, 4], F32, kind="ExternalInput").ap()
    qkw_d = nc.dram_tensor("qkw", [P, 4], F32, kind="ExternalInput").ap()
    onesd_d = nc.dram_tensor("onesd", [P, P], BF16, kind="ExternalInput").ap()
    ones2_d = nc.dram_tensor("ones2", [2, P], F32, kind="ExternalInput").ap()
    out_d = nc.dram_tensor("out", [n, C], BF16, kind="ExternalOutput").ap()

    with tile.TileContext(nc) as tc, ExitStack() as ctx:
        con = ctx.enter_context(tc.tile_pool(name="con", bufs=1))
        wp = ctx.enter_context(tc.tile_pool(name="wp", bufs=1))
        qk = ctx.enter_context(tc.tile_pool(name="qk", bufs=1))
        vp = ctx.enter_context(tc.tile_pool(name="vp", bufs=1))
        xp = ctx.enter_context(tc.tile_pool(name="xp", bufs=1))
        sqp = ctx.enter_context(tc.tile_pool(name="sqp", bufs=4))
        rp = ctx.enter_context(tc.tile_pool(name="rp", bufs=4))
        ps = ctx.enter_context(tc.tile_pool(name="ps", bufs=2, space="PSUM"))
        ob = ctx.enter_context(tc.tile_pool(name="ob", bufs=4, space="PSUM"))
        ptp = ctx.enter_context(tc.tile_pool(name="ptp", bufs=20))
        atp = ctx.enter_context(tc.tile_pool(name="atp", bufs=1))
        rp2 = ctx.enter_context(tc.tile_pool(name="rp2", bufs=4))
        osp = ctx.enter_context(tc.tile_pool(name="osp", bufs=4))

        # ---- constants ----
        # block-diag ones (64x64 blocks) as lhsT: one matmul broadcasts each
        # head's partition-sum of squares back to that head's 64 partitions
        onesd = con.tile([P, P], BF16, tag="onesd")
        nc.sync.dma_start(onesd[:], onesd_d[:])
        # 2-row block mask: row0 -> cols 0:64, row1 -> cols 64:128 (denominator
        # broadcast for both heads of a pair in one K=2 matmul)
        ones2 = con.tile([2, P], F32R, tag="ones2")
        nc.sync.dma_start(ones2[:], ones2_d[:].bitcast(F32R))

        eps_sb = con.tile([P, 1], F32, tag="eps")
        nc.vector.memset(eps_sb[:], EPS)

        bqk_sb = con.tile([P, 4], F32, tag="bqk")
        nc.sync.dma_start(bqk_sb[:], bqk_d[:])
        qkw_sb = con.tile([P, 4], F32, tag="qkw")
        nc.sync.dma_start(qkw_sb[:], qkw_d[:])

        # ---- weight / input loads. Weights ride the ACT HWDGE queue as one
        # merged DMA each (issued before any ACTIVATE contends for that
        # queue); x streams tile-by-tile on the SP queue so the k-outer
        # matmuls start as soon as xT[0] lands.
        wqkM = wp.tile([P, kt * 2 * CL], BF16, tag="wqkM")
        wvM = wp.tile([P, kt * CL], BF16, tag="wvM")
        wprM = wp.tile([P, (CL // P) * C], BF16, tag="wprM")
        xT_sb = [xp.tile([P, n], BF16, tag=f"xt{k}", name=f"xt{k}") for k in range(kt)]
        nc.scalar.dma_start(
            wqkM[:].rearrange("p (k c) -> p k c", k=kt),
            wqk_d[:].rearrange("(k p) c -> p k c", p=P))
        nc.scalar.dma_start(
            wvM[:].rearrange("p (k c) -> p k c", k=kt),
            wv_d[:].rearrange("(k p) c -> p k c", p=P))
        nc.scalar.dma_start(
            wprM[:].rearrange("p (k c) -> p k c", k=CL // P),
            wpr_d[:].rearrange("(k p) c -> p k c", p=P))
        for k in range(kt):
            nc.sync.dma_start(xT_sb[k][:], xT_d[k * P:(k + 1) * P, :])
        def wqk_ap(k, m):
            return wqkM[:, k * 2 * CL + m * P:k * 2 * CL + (m + 1) * P]

        def wv_ap(k):
            return wvM[:, k * CL:(k + 1) * CL]

        def wpr_ap(t, hs):
            return wprM[:, t * C + hs.start:t * C + hs.stop]

        # ---- stage 1a+2: qkT = (x @ w_qk)^T, biased + per-head RMSNorm ----
        # 4 channel blocks of 128: q(heads01), q(heads23), k(heads01), k(heads23)
        # k blocks (m=2,3) first: the attention units need all of k but only
        # one q block each.
        qkB = [qk.tile([P, n], BF16, tag=f"qkB{m}", name=f"qkB{m}") for m in range(4)]

        def emit_evac_sq(m, j, acc):
            js = slice(j * 512, (j + 1) * 512)
            # evacuate with bias add (DVE: f32r out), square with bias (ACT)
            nc.vector.tensor_scalar_add(qkB[m][:, js], acc[:],
                                        bqk_sb[:, m:m + 1])
            sq = sqp.tile([P, 512], BF16, tag="sq", name="sq", bufs=6)
            nc.scalar.activation(sq[:], acc[:], AF.Square,
                                 bias=bqk_sb[:, m:m + 1])
            return sq

        def emit_norm_chain(m, j, sq):
            js = slice(j * 512, (j + 1) * 512)
            # per-head sum of squares, broadcast to the head's partitions
            ssq = ob.tile([P, 512], F32, tag="ob", name="ssq", space="PSUM")
            nc.tensor.matmul(ssq[:], onesd[:], sq[:], start=True, stop=True)
            rms = rp.tile([P, 512], F32, tag="rms", name="rms")
            nc.scalar.activation(rms[:], ssq[:], AF.Sqrt,
                                 scale=1.0 / D, bias=eps_sb[:, 0:1])
            rec = rp.tile([P, 512], F32, tag="rec", name="rec")
            nc.vector.reciprocal_approx_fast(rec[:], rms[:])
            # qkB = (qkB * qk_weight_col) * (1/rms), in place
            nc.vector.scalar_tensor_tensor(
                qkB[m][:, js], qkB[m][:, js], qkw_sb[:, m:m + 1],
                rec[:], op0=ALU.mult, op1=ALU.mult)

        # pending (m, j, sq) norm chains: the ssq matmul of iteration t is
        # emitted during iteration t+1 so the PE never waits on ACT's Square
        pending = []

        # first channel block runs the contraction loop k-OUTER with 4 live
        # accumulators: matmuls start as soon as xT[k] arrives instead of
        # idling until the whole x transfer completes
        accs = [ob.tile([P, 512], F32, tag="ob", name=f"acc2_{j}",
                        space="PSUM") for j in range(n // 512)]
        korder = list(range(kt))  # DMA arrival order (single input queue)
        for ki, k in enumerate(korder):
            for j in range(n // 512):
                nc.tensor.matmul(
                    accs[j][:], wqk_ap(k, 2),
                    xT_sb[k][:, j * 512:(j + 1) * 512],
                    start=(ki == 0), stop=(ki == kt - 1))
        for j in range(n // 512):
            pending.append((2, j, emit_evac_sq(2, j, accs[j])))

        for m in (3, 0, 1):
            for j in range(n // 512):
                js = slice(j * 512, (j + 1) * 512)
                acc = ps.tile([P, 512], F32, tag="sp", name="acc")
                for k in range(kt):
                    nc.tensor.matmul(
                        acc[:], wqk_ap(k, m), xT_sb[k][:, js],
                        start=(k == 0), stop=(k == kt - 1))
                if pending:
                    emit_norm_chain(*pending.pop(0))
                pending.append((m, j, emit_evac_sq(m, j, acc)))

        # ---- stage 1b: v natural [n, HL, VW] with ones column per head ----
        # remaining norm chains drain between v blocks
        v_aug = [vp.tile([P, HL, VW], BF16, tag=f"va{i}", name=f"va{i}") for i in range(nb)]
        for i in range(nb):
            vacc = ob.tile([P, CL], F32, tag="ob", name="vacc", space="PSUM")
            for k in range(kt):
                nc.tensor.matmul(
                    vacc[:], xT_sb[k][:, i * P:(i + 1) * P], wv_ap(k),
                    start=(k == 0), stop=(k == kt - 1))
            if pending:
                emit_norm_chain(*pending.pop(0))
            nc.vector.memset(v_aug[i][:, :, D:VW], 1.0)
            nc.scalar.activation(
                v_aug[i][:, :, 0:D],
                vacc[:].rearrange("p (h x) -> p h x", h=HL), AF.Copy)

        # ---- stage 3 + 4: attention per (query block, head pair) + proj ----
        attnT = [atp.tile([P, n], BF16, tag=f"at{t}", name=f"at{t}") for t in range(CL // P)]
        # units are (query-block, head-PAIR): the even head's kT/qT live at
        # partition offset 0, the odd head's at offset 64, so consecutive S
        # matmuls target alternating PE array row groups and run concurrently
        units = [(j, hp) for j in range(nj) for hp in range(HL // HPB)]

        def emit_s_pair(u, i):
            """S^T matmuls for both heads of the pair at key-block i + exp."""
            j, hp = u
            js = slice(j * nq, (j + 1) * nq)
            qm, km = hp, 2 + hp
            s2 = ps.tile([P, 2 * nq], F32, tag="sp", name="s2")
            for sub in range(HPB):
                pr = slice(sub * D, (sub + 1) * D)
                nc.tensor.matmul(
                    s2[:, sub * nq:(sub + 1) * nq],
                    qkB[km][pr, i * P:(i + 1) * P], qkB[qm][pr, js],
                    start=True, stop=True)
            pt = ptp.tile([P, 2 * nq], BF16, tag="pt", name="pt")
            if (i % 4) in DVE_EXP_MOD:
                # Schraudolph exp on the DVE: round(y*A+B) as int16 == bf16 bits
                nc.vector.tensor_scalar(
                    pt[:].bitcast(I16), s2[:],
                    float(SCALE * A16), float(B16), ALU.mult, ALU.add)
            else:
                nc.scalar.activation(pt[:], s2[:], AF.Exp, scale=SCALE)
            return pt

        def emit_av(u, oas, pts, i):
            j, hp = u
            for sub in range(HPB):
                h = hp * HPB + sub
                nc.tensor.matmul(
                    oas[sub][:], v_aug[i][:, h, :],
                    pts[i][:, sub * nq:(sub + 1) * nq],
                    start=(i == 0), stop=(i == nb - 1))

        def emit_norm(u, oas):
            j, hp = u
            js = slice(j * nq, (j + 1) * nq)
            # per-head denominator row -> K=1 broadcast matmul -> reciprocal
            for sub in range(HPB):
                pr = slice(sub * D, (sub + 1) * D)
                sums = rp2.tile([1, nq], F32R, tag="sums", name="sums")
                nc.vector.tensor_copy(sums[:], oas[sub][D:VW, :])
                bc = ob.tile([D, nq], F32, tag="ob", name="bc", space="PSUM")
                nc.tensor.matmul(bc[:], ones2[0:1, 0:D], sums[:],
                                 start=True, stop=True)
                rec = rp2.tile([D, nq], F32, tag="recw", name="recw")
                nc.vector.reciprocal_approx_fast(rec[:], bc[:])
                nc.vector.tensor_mul(attnT[hp][pr, js], oas[sub][0:D, :],
                                     rec[:])

        def emit_proj(j):
            for jj, j2 in enumerate(range(j * (nq // P), (j + 1) * (nq // P))):
                for half in range(2):
                    hs = slice(half * 512, (half + 1) * 512)
                    acc = ob.tile([P, 512], F32, tag="ob", name="acc",
                                  space="PSUM")
                    for t in range(CL // P):
                        nc.tensor.matmul(
                            acc[:], attnT[t][:, j2 * P:(j2 + 1) * P],
                            wpr_ap(t, hs), start=(t == 0),
                            stop=(t == CL // P - 1))
                    ost = osp.tile([P, 512], BF16, tag="ost", name="ost")
                    # alternate evacuation + output queue between ACT and DVE
                    if (jj + half) % 2 == 0:
                        nc.scalar.copy(ost[:], acc[:])
                        nc.scalar.dma_start(out_d[j2 * P:(j2 + 1) * P, hs],
                                            ost[:])
                    else:
                        nc.vector.tensor_copy(ost[:], acc[:])
                        nc.sync.dma_start(out_d[j2 * P:(j2 + 1) * P, hs],
                                          ost[:])

        # software pipeline: S pairs of unit u+1 interleave with AV of unit u;
        # each finished query block's projection is deferred into the NEXT
        # iteration so the PE doesn't sit behind the norm chain's DVE latency
        prev = None   # (unit, pts)
        deferred_proj = None
        for idx in range(len(units) + 1):
            cur = units[idx] if idx < len(units) else None
            pts = []
            oas_prev = None
            if prev is not None:
                oas_prev = [ob.tile([VW, nq], F32, tag="ob", bufs=4,
                                    space="PSUM", name=f"oa{s_}")
                            for s_ in range(HPB)]
            BLK = 4
            for ib in range(nb // BLK):
                if cur is not None:
                    for i in range(ib * BLK, (ib + 1) * BLK):
                        pts.append(emit_s_pair(cur, i))
                if ib == 0 and deferred_proj is not None:
                    emit_proj(deferred_proj)
                    deferred_proj = None
                if prev is not None:
                    for i in range(ib * BLK, (ib + 1) * BLK):
                        emit_av(prev[0], oas_prev, prev[1], i)
            if prev is not None:
                emit_norm(prev[0], oas_prev)
                jprev, hpprev = prev[0]
                if hpprev == HL // HPB - 1:
                    deferred_proj = jprev
            prev = (cur, pts) if cur is not None else None
        if deferred_proj is not None:
            emit_proj(deferred_proj)

    nc.compile()
    return nc


_NC_CACHE = {}


def _get_nc(n=N, nq=NQ):
    key = (n, nq)
    if key not in _NC_CACHE:
        _NC_CACHE[key] = build(n, nq)
    return _NC_CACHE[key]


def make_in_maps(x, w_qkv, b_qkv, q_w, k_w, w_proj, b_proj):
    """Shard full inputs into per-core in_maps (host side)."""
    bf = ml_dtypes.bfloat16
    in_maps = []
    for cid in range(NCORES):
        b, g = cid // GROUPS, cid % GROUPS
        c0 = g * CL
        xT = np.ascontiguousarray(x[b].T)
        w_qk = np.ascontiguousarray(
            np.concatenate([w_qkv[:, c0:c0 + CL],
                            w_qkv[:, C + c0:C + c0 + CL]], axis=1))
        w_v = np.ascontiguousarray(w_qkv[:, 2 * C + c0:2 * C + c0 + CL])
        w_pr = np.ascontiguousarray(w_proj[c0:c0 + CL, :])
        b_qk = np.stack([b_qkv[c0 + m * P:c0 + (m + 1) * P] for m in range(2)]
                        + [b_qkv[C + c0 + m * P:C + c0 + (m + 1) * P]
                           for m in range(2)], axis=1)
        qkw = np.stack([np.tile(q_w, HPB), np.tile(q_w, HPB),
                        np.tile(k_w, HPB), np.tile(k_w, HPB)], axis=1)
        onesd = np.zeros((P, P), np.float32)
        onesd[:D, :D] = 1.0
        onesd[D:, D:] = 1.0
        ones2 = np.zeros((2, P), np.float32)
        ones2[0, :D] = 1.0
        ones2[1, D:] = 1.0
        in_maps.append({
            "xT": xT.astype(bf),
            "w_qk": w_qk.astype(bf),
            "w_v": w_v.astype(bf),
            "w_pr": w_pr.astype(bf),
            "b_qk": np.ascontiguousarray(b_qk).astype(np.float32),
            "qkw": np.ascontiguousarray(qkw).astype(np.float32),
            "onesd": onesd.astype(bf),
            "ones2": ones2,
        })
    return in_maps


def kernel(x, w_qkv, b_qkv, q_w, k_w, w_proj, b_proj, _trace=False):
    x = np.asarray(x, np.float32)
    w_qkv = np.asarray(w_qkv, np.float32)
    b_qkv = np.asarray(b_qkv, np.float32)
    w_proj = np.asarray(w_proj, np.float32)
    b_proj = np.asarray(b_proj, np.float32)
    n = x.shape[1]
    nc = _get_nc(n, NQ)
    in_maps = make_in_maps(x, w_qkv, np.asarray(b_qkv, np.float32),
                           np.asarray(q_w, np.float32),
                           np.asarray(k_w, np.float32), w_proj, b_proj)
    res = run_bass_kernel_spmd(nc, in_maps, core_ids=list(range(NCORES)),
                               trace=_trace)
    # v-bias passes through softmax additively; add its projection (and
    # b_proj) once per batch on the host
    bias_row = (b_qkv[2 * C:] @ w_proj + b_proj).astype(np.float32)
    # TP unshard: sum the 4 head-group partials per batch, stack batches
    out = np.stack([
        sum(res.results[b * GROUPS + g]["out"].astype(np.float32)
            for g in range(GROUPS)) + bias_row
        for b in range(B)
    ]).astype(np.float32)
    if _trace:
        return out, res
    return out
